# revision 21
# baseline (speedup 1.0000x reference)
"""Haar wavelet (2x2 stride-2, per-channel) Trainium2 Bass kernel.

Full input x: (8, 64, 512, 512) f32 -> full output (8, 256, 256, 256) f32.
Sharding: pure data parallel over batch -- core i processes x[i].

Per-core layout (C=64 channels, H=W=512), v6:
  - Block = KC=2 channels x full height. Partition p = k*64 + b holds
    input rows 8b..8b+7 of channel c0+k: one 16 KB contiguous DRAM run
    per partition per load.
  - ACT engine halves the tile in place (activation Copy, scale=0.5),
    freeing the DVE of one full pass.
  - DVE vertical butterfly (2 ops, FD 2048): s = top+bot, d = bot-top,
    written interleaved into one mid tile m = (v, a, w).
  - DVE horizontal butterfly (2 ops, FD 2048): the (s,d) interleave
    makes (ll,lh) = even+odd and (hl,hh) = odd-even each a single
    tensor_tensor over v in {s,d}.
  - Store: partition p holds 4 output rows x 4 subbands of one channel:
    4 runs of 4 KB contiguous DRAM each (one DMA per channel; DMA APs
    cap at 3 dims).
  - DMA is split SYMMETRICALLY across the two HWDGE rings: block i's
    load goes to ring i%2, its stores to ring (i+1)%2, so each ring
    carries a balanced load/store mix in FIFO order. With all loads on
    one ring and all stores on the other (v3), store service lagged
    mid-kernel (~155 GB/s vs loads' 180+) and ~20 MiB of stores drained
    in a 50 us tail; balanced rings keep both directions flowing.
  - Loads prefetch 4 blocks ahead; each block's stores are emitted right
    after its compute so store traffic starts as early as possible.
Engine roles: ACT = halve + half the DMA, SP = other half, DVE = butterflies.
Measured per-core HBM envelope on this pool: pure reads ~427 GB/s, pure
writes ~365 GB/s, sustained mixed ~330-375 GB/s (shared-chip ambient noise
gives +-35 us run-to-run). Roofline: 128 MiB / ~350 GB/s = ~375 us; DVE is
4 ops x (2048+151) cyc x 32 blocks / 0.96 GHz = ~293 us (hidden under DMA).
Measured: 366-417 us (vs 447 us baseline same-session).
"""

import sys

if "/opt/trn_rl_repo" not in sys.path:
    sys.path.insert(0, "/opt/trn_rl_repo")

from contextlib import ExitStack

import numpy as np

import concourse.bass as bass
import concourse.tile as tile
from concourse import bacc
from concourse import mybir
from concourse.bass_utils import run_bass_kernel_spmd

N_CORES = 8
C, H, W = 64, 512, 512
F32 = mybir.dt.float32
BF16 = mybir.dt.bfloat16
ADD = mybir.AluOpType.add
SUB = mybir.AluOpType.subtract

_CACHED = {}


def _build(C=C, H=H, W=W, KC=2, R=8, PF=4):
    HO, WO = H // 2, W // 2
    A = R // 2               # output rows per partition
    PB = H // R              # partitions per channel (64)
    assert KC * PB == 128
    n_blocks = C // KC
    FD = R * W               # free-dim elems per partition (4096)

    nc = bacc.Bacc("TRN2", target_bir_lowering=False, debug=False)
    x = nc.dram_tensor("x", [C, H, W], F32, kind="ExternalInput").ap()
    out = nc.dram_tensor("out", [4 * C, HO, WO], F32, kind="ExternalOutput").ap()

    with tile.TileContext(nc) as tc, ExitStack() as ctx:
        xpool = ctx.enter_context(tc.tile_pool(name="xp", bufs=PF + 2))
        mpool = ctx.enter_context(tc.tile_pool(name="mp", bufs=2))
        rpool = ctx.enter_context(tc.tile_pool(name="rp", bufs=4))

        rings = [nc.scalar, nc.sync]
        xts, rts = {}, {}

        def emit_load(i):
            c0 = i * KC
            xt = xpool.tile([128, FD], F32)
            src = x[c0 : c0 + KC].rearrange("k (b f) w -> (k b) f w", f=R)
            rings[i % 2].dma_start(xt[:].rearrange("p (f w) -> p f w", w=W), src)
            xts[i] = xt

        def emit_compute(i):
            xt = xts.pop(i)

            # ---- halve in place on ACT (activation Copy, scale 0.5)
            nc.scalar.mul(xt[:], xt[:], 0.5)

            x4 = xt[:].rearrange("p (a t w) -> p a t w", t=2, w=W)
            top, bot = x4[:, :, 0, :], x4[:, :, 1, :]

            # ---- vertical butterfly (DVE), s/d interleaved
            m_t = mpool.tile([128, 2 * A * W], F32)
            mv = m_t[:].rearrange("p (v a w) -> p v a w", v=2, a=A)
            nc.vector.tensor_tensor(mv[:, 0], top, bot, ADD)   # s
            nc.vector.tensor_tensor(mv[:, 1], bot, top, SUB)   # d

            # ---- horizontal butterfly (DVE), 2 fused ops
            m5 = m_t[:].rearrange("p (v a j t) -> p v a j t", v=2, a=A, t=2)
            ev, od = m5[:, :, :, :, 0], m5[:, :, :, :, 1]
            rt = rpool.tile([128, 4 * A * WO], F32)
            r4 = rt[:].rearrange("p (u a j) -> p u a j", u=4, a=A)
            nc.vector.tensor_tensor(r4[:, 0:2], ev, od, ADD)   # ll, lh
            nc.vector.tensor_tensor(r4[:, 2:4], od, ev, SUB)   # hl, hh
            rts[i] = rt

        def emit_store(i):
            c0 = i * KC
            rt = rts.pop(i)
            for k in range(KC):
                ck = c0 + k
                dst = out[4 * ck : 4 * ck + 4].rearrange(
                    "q (b r) w -> b q (r w)", r=A
                )
                src = rt[k * PB : (k + 1) * PB].rearrange("b (q f) -> b q f", q=4)
                rings[(i + 1) % 2].dma_start(dst, src)

        for i in range(PF):
            emit_load(i)
        for i in range(n_blocks):
            if i + PF < n_blocks:
                emit_load(i + PF)
            emit_compute(i)
            emit_store(i)
    nc.compile()
    return nc


def _get_nc():
    if "nc" not in _CACHED:
        _CACHED["nc"] = _build()
    return _CACHED["nc"]


def _run(x, **kwargs):
    x = np.ascontiguousarray(np.asarray(x), dtype=np.float32)
    assert x.shape == (N_CORES, C, H, W), x.shape
    nc = _get_nc()
    in_maps = [{"x": np.ascontiguousarray(x[i])} for i in range(N_CORES)]
    res = run_bass_kernel_spmd(nc, in_maps, core_ids=list(range(N_CORES)), **kwargs)
    out = np.stack([res.results[i]["out"] for i in range(N_CORES)], axis=0)
    return out, res


def kernel(x):
    return _run(x)[0]
